# revision 33
# baseline (speedup 1.0000x reference)
"""Trainium2 Bass kernel for a single-head attention block (B=8, S=2048, D=512, dk=dv=64).

Sharding: one batch element per NeuronCore (8 cores, data parallel).

Per-core algorithm (batch b), all in "transposed" layouts chosen so that every
matmul contraction runs over the SBUF partition axis:

  host:   qkT = [q[b].T d-chunks (4x128); k[b].T d-chunks]   [8, 128, 2048]
          vT  = v[b].T d-chunks                              [4, 128, 2048]
  DMA:    per s-quarter 3D-AP transfers (0.5 MB each) on 3 engine queues,
          ordered so column 0's operands land first
  proj:   col-packed weights-stationary matmuls: Wq chunk on PE columns
          0:64, Wk chunk on 64:128 (tile_position), streaming qT and kT
          quarters concurrently -> pj = [qp; kp] in one PSUM bank;
          vp[t,dv] natural layout via data-stationary matmuls
  scores: sT[t,s] = sum_d kp[d,t] qp[d,s], t-chunks of 128, row-packed in
          pairs on the PE (tile_position (0,0)/(64,0), K=64 each)
  P       = exp(sT * 1/8) on ACT (scale fused; no max-subtraction: scores
          are O(5) so fp32 exp is exact-safe).  Causal mask applied AFTER
          exp as a gpsimd affine_select (fill 0 where t > s), which also
          lets the diagonal pair's exp skip fully-masked s-quarters.
  AV:     avT[dv,s] = sum_t vpe[t,dv] P[t,s], vpe = [(vp+bv)*E | E] with
          E[t] = exp(pad[t]); row 64 of avT is the softmax denominator
  out:    out[s,dv] = avT[dv,s] / (avT[64,s] + 1e-10)  (PE transpose + DVE)

Matmul dtype is fp16 (10-bit mantissa): ~5e-4 rel error vs the fp32
reference, 1 cycle/row on the PE, 1-pass weight loads, half-size DMA.
"""

import numpy as np

B, S, D, DK, DV = 8, 2048, 512, 64, 64
NCORES = 8
SC = 512              # s-chunk (attention column) width
NSC = S // SC         # 4
NT = S // 128         # 16 t-chunks

CFG = dict(
    qk_dtype="float16",    # q/k projections + scores matmul precision
    v_dtype="float16",     # v projection, P (attention weights), AV matmul
    colpack_proj=True,     # col-packed weights-stationary qk projection
    gpsimd_mask=True,      # causal mask via affine_select on P (else DVE add)
    sliced_exp=True,       # skip exp on fully-masked diag s-quarters
    warm_n=12,             # warm-up matmuls during initial DMA wait
    warm_bridge=6,         # N=512 warm matmuls per column boundary
    col_wait_us=(6.5, 13.5, 18.5, 23.5),  # modeled input-quarter arrival times
    trace=False,           # collect NTFF profile (set by test.py)
)

_prog = None


def _build_program():
    from contextlib import ExitStack

    import concourse.bass as bass
    import concourse.mybir as mybir
    import concourse.tile as tile
    from concourse import bacc

    f32 = mybir.dt.float32
    qkdt = getattr(mybir.dt, CFG["qk_dtype"])
    vdt = getattr(mybir.dt, CFG["v_dtype"])

    nc = bacc.Bacc(
        trn_type="TRN2",
        target_bir_lowering=False,
        debug=False,
        num_devices=NCORES,
    )

    # [q, p, (c, w)]: s-quarter q of all 8 qk D-chunks (0:4 = q.T, 4:8 = k.T),
    # host-packed as the exact SBUF image so each quarter is one contiguous
    # 2D DMA with 8 KB partition lines
    qkT_d = nc.dram_tensor("qkT", [4, 128, 8 * SC], qkdt, kind="ExternalInput").ap()
    # [q, p, (c, w)]: t-quarter q of the 4 v D-chunks, same packing
    vT_d = nc.dram_tensor("vT", [4, 128, 4 * SC], vdt, kind="ExternalInput").ap()
    # weights packed [p, (dc, m)]: W[128*dc + p, m] at [p, 64*dc + m]
    wq_d = nc.dram_tensor("wq", [128, 256], qkdt, kind="ExternalInput").ap()
    wk_d = nc.dram_tensor("wk", [128, 256], qkdt, kind="ExternalInput").ap()
    wv_d = nc.dram_tensor("wv", [128, 256], vdt, kind="ExternalInput").ap()
    bias_qk_d = nc.dram_tensor("bias_qk", [128, 1], f32, kind="ExternalInput").ap()
    bvrow_d = nc.dram_tensor("bvrow", [1, DV], f32, kind="ExternalInput").ap()
    padT_d = nc.dram_tensor("padT", [128, NT], f32, kind="ExternalInput").ap()
    out_d = nc.dram_tensor("out", [S, DV], f32, kind="ExternalOutput").ap()

    Exp = mybir.ActivationFunctionType.Exp

    with tile.TileContext(nc) as tc:
        with ExitStack() as ctx:
            const = ctx.enter_context(tc.tile_pool(name="const", bufs=1))
            pp = ctx.enter_context(tc.tile_pool(name="pp", bufs=4))
            sbw = ctx.enter_context(tc.tile_pool(name="sbw", bufs=4))
            ps_qk = ctx.enter_context(tc.tile_pool(name="ps_qk", bufs=2, space="PSUM"))
            ps_pj = ctx.enter_context(tc.tile_pool(name="ps_pj", bufs=1, space="PSUM"))
            ps_pjv = ctx.enter_context(tc.tile_pool(name="ps_pjv", bufs=1, space="PSUM"))
            ps_av = ctx.enter_context(tc.tile_pool(name="ps_av", bufs=2, space="PSUM"))

            # warm-matmul source: zero tile with no DMA dependency, memset on
            # the otherwise-idle DVE so the first warm matmul can issue
            # immediately after the engine preamble
            warmsrc = const.tile([128, SC], qkdt, tag="warmsrc")
            nc.vector.memset(warmsrc[:], 0.0)

            # ---- tiny consts first (weights needed before any compute) ----
            wq = const.tile([128, 256], qkdt, tag="wq")
            nc.gpsimd.dma_start(out=wq[:], in_=wq_d[:])
            wk = const.tile([128, 256], qkdt, tag="wk")
            nc.scalar.dma_start(out=wk[:], in_=wk_d[:])
            wv = const.tile([128, 256], vdt, tag="wv")
            nc.gpsimd.dma_start(out=wv[:], in_=wv_d[:])
            padT = const.tile([128, NT], f32, tag="padT")
            nc.gpsimd.dma_start(out=padT[:], in_=padT_d[:])
            bias_qk = const.tile([128, 1], f32, tag="bias_qk")
            nc.gpsimd.dma_start(out=bias_qk[:], in_=bias_qk_d[:])
            # bv broadcast across partitions (bv varies along the free axis of vp)
            bvb = const.tile([128, DV], f32, tag="bvb")
            nc.gpsimd.dma_start(out=bvb[:], in_=bvrow_d.partition_broadcast(128))

            # resident input quarters: qkt_q[q] holds the s in [512q, 512q+512)
            # slice of all 8 qk chunks; vt_q[q] the same t-range of 4 v chunks
            qkt_q = [
                const.tile([128, 8 * SC], qkdt, tag=f"qkt{q}", name=f"qkt{q}")
                for q in range(NSC)
            ]
            vt_q = [
                const.tile([128, 4 * SC], vdt, tag=f"vt{q}", name=f"vt{q}")
                for q in range(NSC)
            ]
            for q in range(NSC):
                # single-queue DMA tops out near ~110 GB/s, so spread each
                # quarter across all three queues: q-chunks on sync HWDGE,
                # k-chunks on scalar HWDGE, v on gpsimd SWDGE (~2 MB each)
                nc.sync.dma_start(
                    out=qkt_q[q][:, 0 : 4 * SC], in_=qkT_d[q, :, 0 : 4 * SC]
                )
                nc.scalar.dma_start(
                    out=qkt_q[q][:, 4 * SC : 8 * SC], in_=qkT_d[q, :, 4 * SC : 8 * SC]
                )
                nc.gpsimd.dma_start(out=vt_q[q][:], in_=vT_d[q])

            # E[t] = exp(pad[t]) (also triggers the ACT table load early)
            E = const.tile([128, NT], f32, tag="E")
            nc.scalar.activation(E[:], padT[:], Exp)

            # identity for PE transposes
            ident = const.tile([128, 128], f32, tag="ident")
            nc.gpsimd.memset(ident[:], 0.0)
            nc.gpsimd.affine_select(
                out=ident[:],
                in_=ident[:],
                compare_op=mybir.AluOpType.not_equal,
                fill=1.0,
                base=0,
                pattern=[[-1, 128]],
                channel_multiplier=1,
            )

            # PE warmup during the initial DMA wait: N=256 matmuls on the
            # weight tiles keep the HAM activity window busy so real matmuls
            # start at the full 2.4 GHz clock
            warm = ps_pj.tile([128, SC], f32, tag="pj")

            def warm_mm(n, w=256):
                for _ in range(n):
                    nc.tensor.matmul(
                        warm[:, 0:w],
                        warmsrc[:, 0:128],
                        warmsrc[:, 0:w],
                        start=True,
                        stop=True,
                    )

            warm_mm(CFG["warm_n"])

            # per-s-chunk projections: qkp = [qp; kp], kqp = [kp; qp] (the swap
            # provides both PE partition placements for row-packed score MMs)
            qkp = [
                const.tile([128, SC], qkdt, tag=f"qkp{i}", name=f"qkp{i}")
                for i in range(NSC)
            ]
            kqp = [
                const.tile([128, SC], qkdt, tag=f"kqp{i}", name=f"kqp{i}")
                for i in range(NSC)
            ]
            # vpe4[sc][:, 65r:65r+65] = [(vp_j + bv) * E_j | E_j], j = 4sc + r
            vpe4 = [
                const.tile(
                    [128, 4 * (DV + 1)], vdt, tag=f"vpe4_{i}", name=f"vpe4_{i}"
                )
                for i in range(NSC)
            ]

            post_queue = []

            def emit_post():
                if not post_queue:
                    return
                psc, pav = post_queue.pop(0)
                # transpose avT back and normalize: out = avT[:64]/(avT[64]+eps)
                avsb = sbw.tile([DV + 1, SC], f32, tag="avsb", name="avsb")
                nc.vector.tensor_copy(avsb[:], pav[0 : DV + 1, :])
                tpb = ps_av.tile([128, SC], f32, tag="av", name="tpb")
                for m in range(SC // 128):
                    nc.tensor.transpose(
                        tpb[:, bass.ds(65 * m, DV + 1)],
                        avsb[:, bass.ts(m, 128)],
                        ident[0 : DV + 1, 0 : DV + 1],
                    )
                tpv = tpb[:, 0 : 4 * 65].rearrange("p (m c) -> p m c", c=DV + 1)
                rcp = sbw.tile([128, 4], f32, tag="rcp", name="rcp")
                nc.vector.tensor_scalar_add(
                    rcp.rearrange("p (m c) -> p m c", c=1),
                    tpv[:, :, DV : DV + 1],
                    1e-10,
                )
                nc.vector.reciprocal(rcp[:], rcp[:])
                ot = sbw.tile([128, 4 * DV], f32, tag="ot", name="ot")
                otv = ot.rearrange("p (m c) -> p m c", c=DV)
                nc.vector.tensor_mul(
                    otv,
                    tpv[:, :, 0:DV],
                    rcp.rearrange("p (m c) -> p m c", c=1).broadcast_to(
                        [128, 4, DV]
                    ),
                )
                nc.sync.dma_start(
                    out=out_d[bass.ds(psc * SC, SC), :].rearrange(
                        "(m p) v -> p m v", p=128
                    ),
                    in_=otv,
                )

            for sc in range(NSC):
                qv = qkt_q[sc].rearrange("p (c w) -> p c w", w=SC)
                # Tell the scheduler's cost-model sim when this column's input
                # quarter actually lands (its DMA model is far too optimistic
                # under 8-core HBM contention).  Without this it fixes engine
                # orders that put later columns' data-stalled matmuls AHEAD of
                # this column's softmax chain in the PE FIFO (head-of-line
                # blocking: col0's exp waited on col1's DMA).
                T = CFG["col_wait_us"][sc]
                if sc > 0:
                    # bridge the tail of the data wait so the HAM clock stays
                    # warm (PE re-throttles to 1.2 GHz after ~3.4us idle)
                    tc.tile_set_cur_wait((T - 3.0) / 1000.0)
                    warm_mm(CFG["warm_bridge"], w=SC)
                tc.tile_set_cur_wait(T / 1000.0)

                # ---- q/k projections: col-packed, weights stationary.
                # Wq chunk occupies PE columns 0:64 -> qp in PSUM rows 0:64;
                # Wk chunk on columns 64:128 -> kp in rows 64:128. The two
                # chains stream their quarters concurrently (separate XBUSes).
                pj = ps_pj.tile([128, SC], f32, tag="pj")
                tp_q = (0, 0) if CFG["colpack_proj"] else None
                tp_k = (0, 64) if CFG["colpack_proj"] else None
                for dc in range(4):
                    nc.tensor.matmul(
                        pj[0:64, :],
                        wq[:, bass.ts(dc, 64)],
                        qv[:, dc],
                        start=(dc == 0),
                        stop=(dc == 3),
                        tile_position=tp_q,
                    )
                    # skip_group_check: the sim's psum group tracker mishandles
                    # accumulation chains starting at partition base 64; the
                    # per-element pending-zero value semantics remain checked
                    nc.tensor.matmul(
                        pj[64:128, :],
                        wk[:, bass.ts(dc, 64)],
                        qv[:, 4 + dc],
                        start=(dc == 0),
                        stop=(dc == 3),
                        tile_position=tp_k,
                        skip_group_check=True,
                    )
                nc.vector.tensor_scalar_add(qkp[sc][:], pj[:, :], bias_qk[:])
                # partition-swapped copy for the odd row-group score matmuls
                # (fp16 SBUF->SBUF on DVE: hits the 2x/4x packed perf modes)
                nc.vector.tensor_copy(kqp[sc][0:64, :], qkp[sc][64:128, :])
                nc.vector.tensor_copy(kqp[sc][64:128, :], qkp[sc][0:64, :])

                # ---- v projection (natural layout, one accumulation chain)
                pjv = ps_pjv.tile([128, 4 * DV], f32, tag="pjv")
                vv = vt_q[sc].rearrange("p (c w) -> p c w", w=SC)
                for dc in range(4):
                    for r in range(4):
                        nc.tensor.matmul(
                            pjv[:, bass.ts(r, DV)],
                            vv[:, dc, bass.ts(r, 128)],
                            wv[:, bass.ts(dc, 64)],
                            start=(dc == 0 and r == 0),
                            stop=(dc == 3 and r == 3),
                        )
                # vpe4 = [(vp + bv) * E | E], batched over the 4 t-chunks
                vpev = vpe4[sc].rearrange("p (r c) -> p r c", c=DV + 1)[:, :, 0:DV]
                pjvv = pjv.rearrange("p (r c) -> p r c", c=DV)
                Esl = E[:, bass.ts(sc, 4)]
                nc.vector.tensor_add(
                    vpev,
                    pjvv,
                    bvb.rearrange("p (r c) -> p r c", r=1).broadcast_to([128, 4, DV]),
                )
                nc.vector.tensor_mul(vpev, vpev, Esl.broadcast_to([128, 4, DV]))
                nc.vector.tensor_copy(
                    vpe4[sc].rearrange("p (r c) -> p r c", c=DV + 1)[
                        :, :, DV : DV + 1
                    ],
                    Esl.rearrange("p (r c) -> p r c", c=1),
                )

                # ---- attention column sc ----
                av = ps_av.tile([128, SC], f32, tag="av")
                njt = 4 * sc + 4  # active t-chunks in this column
                for g in range(njt // 2):
                    diag0 = g == 2 * sc      # pair covers diag chunks rr=(0,1)
                    diag1 = g == 2 * sc + 1  # pair covers diag chunks rr=(2,3)
                    qk = ps_qk.tile([128, 2 * SC], f32, tag="qk")
                    for r2 in range(2):
                        j = 2 * g + r2
                        jc, jr = j // 4, j % 4
                        if r2 == 1:
                            # odd j: kp/qp copies living at partitions 64:128
                            # run on PE row group 1, concurrent with even j
                            nc.tensor.matmul(
                                qk[:, bass.ts(r2, SC)],
                                qkp[jc][64:128, bass.ts(jr, 128)],
                                kqp[sc][64:128, :],
                                start=True,
                                stop=True,
                                tile_position=(64, 0),
                            )
                        else:
                            nc.tensor.matmul(
                                qk[:, bass.ts(r2, SC)],
                                kqp[jc][0:64, bass.ts(jr, 128)],
                                qkp[sc][0:64, :],
                                start=True,
                                stop=True,
                                tile_position=(0, 0),
                            )
                    if g == 0:
                        emit_post()
                    P = pp.tile([128, 2 * SC], vdt, tag="P")
                    if diag1 and CFG["sliced_exp"]:
                        # s in [0, 256) of both chunks is fully below the
                        # causal diagonal: skip the exp there (the select
                        # below zero-fills it, covering the stale region)
                        nc.scalar.activation(
                            P.rearrange("p (r w) -> p r w", w=SC)[:, :, 256:SC],
                            qk.rearrange("p (r w) -> p r w", w=SC)[:, :, 256:SC],
                            Exp,
                            scale=0.125,
                        )
                    else:
                        nc.scalar.activation(P[:], qk[:], Exp, scale=0.125)
                    if CFG["gpsimd_mask"] and (diag0 or diag1):
                        # causal mask: keep P[u, r, w] iff w >= u + 128*(rr0+r)
                        # (t = 128*(4sc+rr0+r) + u <= s = 512*sc + w), else 0.
                        # diag0 masks only w in [0, 256); diag1 masks w in
                        # [0, 256) entirely (memset; also covers the region
                        # sliced_exp skipped) and selects w in [256, 512),
                        # where the keep predicate rebases to the same form.
                        Pv = P.rearrange("p (r w) -> p r w", w=SC)
                        w0 = 0 if diag0 else 256
                        if diag1:
                            nc.gpsimd.memset(Pv[:, :, 0:256], 0.0)
                        nc.gpsimd.affine_select(
                            out=Pv[:, :, w0 : w0 + 256],
                            in_=Pv[:, :, w0 : w0 + 256],
                            compare_op=mybir.AluOpType.is_ge,
                            fill=0.0,
                            base=0,
                            pattern=[[-128, 2], [1, 256]],
                            channel_multiplier=-1,
                        )
                    for r2 in range(2):
                        j = 2 * g + r2
                        nc.tensor.matmul(
                            av[0 : DV + 1, :],
                            vpe4[j // 4][:, bass.ds(65 * (j % 4), DV + 1)],
                            P[:, bass.ts(r2, SC)],
                            start=(j == 0),
                            stop=(j == njt - 1),
                        )

                # postprocess of the previous column is emitted after this
                # column's first matmul group (see emission in the g loop) so
                # the PE can start the new column while DVE drains the old one
                post_queue.append((sc, av))

            emit_post()

    nc.compile()
    return nc


def _in_maps(inputs):
    import ml_dtypes

    np_of = {"bfloat16": ml_dtypes.bfloat16, "float16": np.float16}
    qk_np = np_of.get(CFG["qk_dtype"], np.float32)
    v_np = np_of.get(CFG["v_dtype"], np.float32)
    q = np.asarray(inputs["q"], dtype=np.float32)
    k = np.asarray(inputs["k"], dtype=np.float32)
    v = np.asarray(inputs["v"], dtype=np.float32)
    pad = np.asarray(inputs["pad_masks"], dtype=np.float32)
    Wq = np.asarray(inputs["Wq"], dtype=np.float32)
    Wk = np.asarray(inputs["Wk"], dtype=np.float32)
    Wv = np.asarray(inputs["Wv"], dtype=np.float32)
    bq = np.asarray(inputs["bq"], dtype=np.float32)
    bk = np.asarray(inputs["bk"], dtype=np.float32)
    bv = np.asarray(inputs["bv"], dtype=np.float32)

    # weights packed [p, (dc, m)]: W[128*dc + p, m] at [p, 64*dc + m]
    def packw(W, dt):
        return np.ascontiguousarray(
            W.reshape(4, 128, 64).transpose(1, 0, 2).reshape(128, 256)
        ).astype(dt)

    wq_p = packw(Wq, qk_np)
    wk_p = packw(Wk, qk_np)
    wv_p = packw(Wv, v_np)
    bias_qk = np.ascontiguousarray(np.concatenate([bq, bk]).reshape(128, 1))

    maps = []
    for b in range(B):
        qkcat = np.concatenate(
            [q[b].T.reshape(4, 128, S), k[b].T.reshape(4, 128, S)], axis=0
        )  # [c=8, p=128, s=2048]
        # SBUF image per s-quarter: [q, p, (c, w)]
        qk_img = (
            qkcat.reshape(8, 128, 4, SC).transpose(2, 1, 0, 3).reshape(4, 128, 8 * SC)
        )
        v_img = (
            v[b].T.reshape(4, 128, 4, SC).transpose(2, 1, 0, 3).reshape(4, 128, 4 * SC)
        )
        maps.append(
            {
                "qkT": np.ascontiguousarray(qk_img).astype(qk_np),
                "vT": np.ascontiguousarray(v_img).astype(v_np),
                "wq": wq_p,
                "wk": wk_p,
                "wv": wv_p,
                "bias_qk": bias_qk,
                "bvrow": np.ascontiguousarray(bv.reshape(1, DV)),
                "padT": np.ascontiguousarray(pad[b, 0].reshape(NT, 128).T),
            }
        )
    return maps


def _install_ntff_shim():
    """This image's antenv lacks axon_hooks; synthesize it so that the
    trace=True NTFF-profiling path of run_bass_kernel_spmd works. No-op on
    images where the module exists or when the boot helper is unavailable."""
    import sys
    import types

    try:
        import antenv.axon_hooks  # noqa: F401

        return
    except ImportError:
        pass
    try:
        sys.path.insert(0, "/root/.axon_site/trn_agent_boot")
        import trn_boot

        hook = trn_boot._ntff_profile_via_ctypes("/opt/axon/libaxon_pjrt.so")
    except Exception:
        hook = None
    mod = types.ModuleType("antenv.axon_hooks")
    mod.get_axon_ntff_profile_hook = lambda: hook
    mod.set_axon_ntff_profile_hook = lambda h: None
    sys.modules["antenv.axon_hooks"] = mod


def kernel(**inputs) -> np.ndarray:
    global _prog
    if _prog is None:
        _prog = _build_program()
    _install_ntff_shim()
    from concourse.bass_utils import run_bass_kernel_spmd

    res = run_bass_kernel_spmd(
        _prog, _in_maps(inputs), core_ids=list(range(NCORES)), trace=CFG["trace"]
    )
    kernel.last_result = res
    return np.stack([res.results[i]["out"] for i in range(NCORES)], axis=0)
